# revision 1
# baseline (speedup 1.0000x reference)
"""MoE ConditionalFeedForward kernel for 8 trn2 NeuronCores.

Strategy: expert parallelism. E=8 experts == 8 cores, so core k owns expert k's
weights (w1[k], w3[k], w2[k]) and processes exactly the (token, slot) pairs
routed to expert k. Routing/gather/scatter run on host; the heavy compute
(3 x C x D x I MACs per core over 1.1 GB of weights) runs on device.

Device math per core (C = padded token capacity, D=2048, I=5632):
  phase 1: hT[i, c] = silu(sum_d w1T[d,i] xT[d,c]) * (sum_d w3T[d,i] xT[d,c])
           (PE matmuls with d on partitions; w1/w3 pre-transposed on host)
  phase 2: y[c, d]  = sum_i hT[i, c] * w2[i, d]
           (PE matmuls with i on partitions; w2 in natural layout)

All weights/activations stream as bf16 (1 PE cycle/row vs 4 for f32; half the
HBM traffic); PSUM accumulation is f32 and the output is f32.
"""

import numpy as np
import ml_dtypes

BF16 = ml_dtypes.bfloat16

# Problem dims (hardcoded per contract; kernel.py must be self-contained).
T, A, E, D, I = 1024, 2, 8, 2048, 5632
N_CORES = 8

_BUILD_CACHE = {}


def _pick_groups(ib):
    """Blocks-per-DMA for the phase-1 (w1/w3) and phase-2 (w2) weight streams."""
    g1 = 2 if ib % 2 == 0 else 1
    g2 = 4 if ib % 4 == 0 else (2 if ib % 2 == 0 else 1)
    return g1, g2


def _pick_npass(d):
    """Split phase 2's D dim into npass passes so the live yT PSUM accumulator
    tags ((d/npass)/128 of them) fit in 8 banks. Prefer double-buffered
    (bufs=2) tags so consecutive passes overlap, then the fewest passes."""
    for bufs in (2, 1):
        for npass in (1, 2, 4, 8, 16):
            ndc = d // npass // 128
            if d % npass == 0 and (d // npass) % 128 == 0 and ndc * bufs <= 8:
                return npass, bufs
    raise ValueError(f"no valid npass for d={d}")


def _build(cap, d=D, i_dim=I):
    """Build + compile the per-core Bass program for token capacity `cap`."""
    key = (cap, d, i_dim)
    if key in _BUILD_CACHE:
        return _BUILD_CACHE[key]

    import concourse.mybir as mybir
    import concourse.tile as tile
    from concourse import bacc

    dt = mybir.dt
    WDT = dt.bfloat16
    F32 = dt.float32

    db = d // 128          # d-chunks (contraction of phase 1)
    ib = i_dim // 128      # i-blocks (contraction of phase 2)
    g1, g2 = _pick_groups(ib)
    ng1, ng2 = ib // g1, ib // g2
    assert cap % 32 == 0 and cap <= 512
    npass, ps_bufs = _pick_npass(d)
    w = d // npass         # output columns per phase-2 pass
    nw = w // 512          # 512-col chunks per pass

    nc = bacc.Bacc("TRN2", target_bir_lowering=False, debug=False,
                   num_devices=N_CORES)

    xgt = nc.dram_tensor("xgt", [128, db * cap], WDT, kind="ExternalInput").ap()
    w1d = nc.dram_tensor("w1d", [ng1, 128, g1 * db * 128], WDT,
                         kind="ExternalInput").ap()
    w3d = nc.dram_tensor("w3d", [ng1, 128, g1 * db * 128], WDT,
                         kind="ExternalInput").ap()
    w2d = nc.dram_tensor("w2d", [npass, 128, ib * w], WDT,
                         kind="ExternalInput").ap()
    # output is y transposed ([D, cap]) so phase 2 can make w2's d-columns the
    # stationary M dim (divides exactly -> no M padding) and write the PSUM
    # [d_block, c] tiles out contiguously; the host untransposes for free.
    yt = nc.dram_tensor("yt", [d, cap], F32, kind="ExternalOutput").ap()

    with tile.TileContext(nc) as tc:
        with (
            tc.tile_pool(name="xpool", bufs=1) as xpool,
            tc.tile_pool(name="w1pool", bufs=3) as w1pool,
            tc.tile_pool(name="w3pool", bufs=3) as w3pool,
            tc.tile_pool(name="w2pool", bufs=3) as w2pool,
            tc.tile_pool(name="hpool", bufs=1) as hpool,
            tc.tile_pool(name="spool", bufs=2) as spool,
            tc.tile_pool(name="opool", bufs=4) as opool,
        ):
            xg = xpool.tile([128, db * cap], WDT)
            # chunked so the first matmuls don't wait on the whole transfer
            xq = max(1, db // 4) * cap
            for q0 in range(0, db * cap, xq):
                nc.sync.dma_start(xg[:, q0:q0 + xq], xgt[:, q0:q0 + xq])
            h = hpool.tile([128, ib * cap], WDT)

            # ---- phase 1: hT blocks ----
            with tc.tile_pool(name="psA", bufs=2, space="PSUM") as psA:
                for g in range(ng1):
                    wt1 = w1pool.tile([128, g1 * db * 128], WDT, tag="w1")
                    nc.sync.dma_start(wt1[:], w1d[g])
                    wt3 = w3pool.tile([128, g1 * db * 128], WDT, tag="w3")
                    nc.sync.dma_start(wt3[:], w3d[g])
                    for s in range(g1):
                        b = g * g1 + s
                        ps1 = psA.tile([128, cap], F32, tag="ps1")
                        ps3 = psA.tile([128, cap], F32, tag="ps3")
                        for do in range(db):
                            lo = (s * db + do) * 128
                            nc.tensor.matmul(
                                ps1[:], wt1[:, lo:lo + 128],
                                xg[:, do * cap:(do + 1) * cap],
                                start=(do == 0), stop=(do == db - 1))
                        for do in range(db):
                            lo = (s * db + do) * 128
                            nc.tensor.matmul(
                                ps3[:], wt3[:, lo:lo + 128],
                                xg[:, do * cap:(do + 1) * cap],
                                start=(do == 0), stop=(do == db - 1))
                        sig = spool.tile([128, cap], F32, tag="sig")
                        nc.scalar.activation(
                            sig[:], ps1[:],
                            mybir.ActivationFunctionType.Sigmoid)
                        m1 = spool.tile([128, cap], F32, tag="m1")
                        nc.vector.tensor_mul(m1[:], sig[:], ps3[:])
                        nc.vector.tensor_mul(
                            h[:, b * cap:(b + 1) * cap], m1[:], ps1[:])

            # ---- phase 2: yT[d, c] = sum_b w2[b, d].T @ hT[b, c] ----
            # stationary = w2 128-column d-blocks (M=128 exact), moving = hT
            # (N=cap). Output tiles are yT blocks, accumulated over all i.
            ndc = w // 128                      # 128-col d-blocks per pass
            # w2 groups: ~8 i-blocks per DMA (1 MB) for full HBM efficiency
            gsz = 8
            groups = [(b0, min(gsz, ib - b0)) for b0 in range(0, ib, gsz)]
            with tc.tile_pool(name="psB", bufs=ps_bufs, space="PSUM") as psB:
                for ph in range(npass):
                    po = {}
                    for dc in range(ndc):
                        po[dc] = psB.tile([128, cap], F32, tag=f"yT{dc}",
                                          name=f"po{dc}")
                    for b0, nb in groups:
                        wt2 = w2pool.tile([128, gsz * w], WDT, tag="w2")
                        nc.sync.dma_start(wt2[:, :nb * w],
                                          w2d[ph][:, b0 * w:(b0 + nb) * w])
                        for s in range(nb):
                            b = b0 + s
                            for dc in range(ndc):
                                lo = s * w + dc * 128
                                nc.tensor.matmul(
                                    po[dc][:],
                                    wt2[:, lo:lo + 128],
                                    h[:, b * cap:(b + 1) * cap],
                                    start=(b == 0), stop=(b == ib - 1))
                    for dc in range(ndc):
                        ot = opool.tile([128, cap], F32, tag="ot")
                        nc.vector.tensor_copy(ot[:], po[dc][:])
                        nc.scalar.dma_start(
                            yt[ph * w + dc * 128:ph * w + dc * 128 + 128, :],
                            ot[:])

    nc.compile()
    _BUILD_CACHE[key] = nc
    return nc


def _pack_w13(wk, d=D, i_dim=I):
    """Host-side relayout of a [I, D] w1/w3 matrix into the pre-transposed
    phase-1 device layout (see _build)."""
    db, ib = d // 128, i_dim // 128
    g1, _ = _pick_groups(ib)
    ng1 = ib // g1
    # [g, s, i_in, do, di] -> [g, di, s, do, i_in]
    return np.ascontiguousarray(
        wk.reshape(ng1, g1, 128, db, 128).transpose(0, 4, 1, 3, 2)
    ).reshape(ng1, 128, g1 * db * 128)


def _pack_w2(w2k, npass, d=D, i_dim=I):
    """[I, D] -> [ph, i_in, b*w + dcol]: per-pass flat block-major layout so
    phase 2 can DMA any run of i-blocks as one big contiguous transfer."""
    ib = i_dim // 128
    w = d // npass
    # [b, i_in, ph, dcol] -> [ph, i_in, b, dcol]
    return np.ascontiguousarray(
        w2k.reshape(ib, 128, npass, w).transpose(2, 1, 0, 3)
    ).reshape(npass, 128, ib * w)


def _prepare(inputs):
    """Host routing + packing. Returns (nc, in_maps, scatter_info)."""
    x = np.asarray(inputs["x"])
    idx = np.asarray(inputs["expert_indices"])
    w1 = np.asarray(inputs["w1"])
    w2 = np.asarray(inputs["w2"])
    w3 = np.asarray(inputs["w3"])

    t, a = idx.shape
    d, i_dim = x.shape[1], w1.shape[1]
    db = d // 128

    # ---- host routing (the "all-to-all") ----
    flat = idx.reshape(-1).astype(np.int64)
    order = np.argsort(flat, kind="stable")          # pair ids grouped by expert
    counts = np.bincount(flat, minlength=E)
    starts = np.concatenate([[0], np.cumsum(counts)])
    cap = max(128, int(-(-counts.max() // 32) * 32))  # round up to mult of 32
    assert cap <= 512, f"capacity {cap} > 512 unsupported"
    npass, _ = _pick_npass(d)

    nc = _build(cap, d, i_dim)

    x_bf = x.astype(BF16)
    in_maps = []
    for k in range(E):
        sel = order[starts[k]:starts[k + 1]] // a      # token ids for expert k
        xg = np.zeros((cap, d), BF16)
        xg[:len(sel)] = x_bf[sel]
        # [c, d] -> [di, do, c]
        xgt = np.ascontiguousarray(
            xg.T.reshape(db, 128, cap).transpose(1, 0, 2)
        ).reshape(128, db * cap)
        w1d_ = _pack_w13(w1[k].astype(BF16), d, i_dim)
        w3d_ = _pack_w13(w3[k].astype(BF16), d, i_dim)
        w2d_ = _pack_w2(w2[k].astype(BF16), npass, d, i_dim)
        in_maps.append({"xgt": xgt, "w1d": w1d_, "w3d": w3d_, "w2d": w2d_})

    return nc, in_maps, (t, a, d, order, counts, starts)


def _scatter(results, scatter_info):
    t, a, d, order, counts, starts = scatter_info
    out_flat = np.zeros((t * a, d), np.float32)
    for k in range(E):
        n_k = int(counts[k])
        if n_k:
            out_flat[order[starts[k]:starts[k] + n_k]] = \
                results[k]["yt"][:, :n_k].T
    return out_flat.reshape(t, a, d)


def kernel(**inputs):
    from concourse.bass_utils import run_bass_kernel_spmd

    nc, in_maps, scatter_info = _prepare(inputs)
    res = run_bass_kernel_spmd(nc, in_maps, core_ids=list(range(N_CORES)))
    return _scatter(res.results, scatter_info)



# revision 4
# speedup vs baseline: 1.0037x; 1.0037x over previous
"""MoE ConditionalFeedForward kernel for 8 trn2 NeuronCores.

Strategy: expert parallelism. E=8 experts == 8 cores, so core k owns expert k's
weights (w1[k], w3[k], w2[k]) and processes exactly the unique (token, expert)
pairs routed to expert k (a token whose two slots pick the same expert is
computed once and scattered to both slots). Routing/gather/scatter run on
host; the heavy compute (3 x C x D x I MACs per core over ~70 MB of bf16
weights) runs on device.

Device math per core (C = padded token capacity, D=2048, I=5632):
  phase 1: hT[i, c] = silu(sum_d w1T[d,i] xT[d,c]) * (sum_d w3T[d,i] xT[d,c])
           (PE matmuls with d on partitions; w1/w3 pre-transposed on host)
  phase 2: y[c, d]  = sum_i hT[i, c] * w2[i, d]
           (PE matmuls with i on partitions; w2 in natural layout)

All weights/activations stream as bf16 (1 PE cycle/row vs 4 for f32; half the
HBM traffic); PSUM accumulation is f32 and the output is f32.

w1/w3 stream on the sync engine's DMA queue; w2 streams on the gpsimd queue
so it prefetches during phase 1 instead of queueing behind the w1/w3 stream,
and phase 2 is never DMA-starved. x and the output use the scalar queue.
"""

import numpy as np
import ml_dtypes

BF16 = ml_dtypes.bfloat16

# Problem dims (hardcoded per contract; kernel.py must be self-contained).
T, A, E, D, I = 1024, 2, 8, 2048, 5632
N_CORES = 8

_BUILD_CACHE = {}


def _phase1_groups(ib):
    """Blocks-per-DMA plan for the w1/w3 streams: small first groups so the
    first matmul's weights land fast, 2-block (1 MB) groups after."""
    plan = [1, 1]
    rem = ib - 2
    while rem > 0:
        g = min(2, rem)
        plan.append(g)
        rem -= g
    out, b0 = [], 0
    for g in plan:
        out.append((b0, g))
        b0 += g
    return out


def _phase2_passes(d):
    """Split phase 2's D dim into uneven passes: 512-wide passes first, a
    small final pass so the end-of-kernel PSUM-evacuation tail is short.
    Constraint: adjacent passes' PSUM tile counts (ndc) sum to <= 8 banks
    with double buffering."""
    if d == 2048:
        widths = [512, 512, 512, 384, 128]
    else:
        assert d % 128 == 0
        widths, r = [], d
        while r > 512:
            widths.append(512)
            r -= 512
        widths.append(r)
    offs, o = [], 0
    for w in widths:
        offs.append(o)
        o += w
    return list(zip(offs, widths))


def _build(cap, d=D, i_dim=I):
    """Build + compile the per-core Bass program for token capacity `cap`."""
    key = (cap, d, i_dim)
    if key in _BUILD_CACHE:
        return _BUILD_CACHE[key]

    import concourse.mybir as mybir
    import concourse.tile as tile
    from concourse import bacc

    dt = mybir.dt
    WDT = dt.bfloat16
    F32 = dt.float32

    db = d // 128          # d-chunks (contraction of phase 1)
    ib = i_dim // 128      # i-blocks (contraction of phase 2)
    groups1 = _phase1_groups(ib)
    passes = _phase2_passes(d)
    assert cap % 4 == 0 and cap <= 512

    nc = bacc.Bacc("TRN2", target_bir_lowering=False, debug=False,
                   num_devices=N_CORES)

    xgt = nc.dram_tensor("xgt", [128, db * cap], WDT, kind="ExternalInput").ap()
    w1d = nc.dram_tensor("w1d", [128, ib * db * 128], WDT,
                         kind="ExternalInput").ap()
    w3d = nc.dram_tensor("w3d", [128, ib * db * 128], WDT,
                         kind="ExternalInput").ap()
    w2d = nc.dram_tensor("w2d", [128, ib * d], WDT, kind="ExternalInput").ap()
    # output is y transposed ([D, cap]) so phase 2 can make w2's d-columns the
    # stationary M dim (divides exactly -> no M padding) and write the PSUM
    # [d_block, c] tiles out contiguously; the host untransposes for free.
    yt = nc.dram_tensor("yt", [d, cap], F32, kind="ExternalOutput").ap()

    with tile.TileContext(nc) as tc:
        with (
            tc.tile_pool(name="xpool", bufs=1) as xpool,
            tc.tile_pool(name="w1pool", bufs=4) as w1pool,
            tc.tile_pool(name="w3pool", bufs=4) as w3pool,
            tc.tile_pool(name="w2pool", bufs=10) as w2pool,
            tc.tile_pool(name="hpool", bufs=1) as hpool,
            tc.tile_pool(name="spool", bufs=2) as spool,
            tc.tile_pool(name="opool", bufs=4) as opool,
        ):
            xg = xpool.tile([128, db * cap], WDT)
            # chunked so the first matmuls don't wait on the whole transfer
            xq = max(1, db // 4) * cap
            for q0 in range(0, db * cap, xq):
                nc.scalar.dma_start(xg[:, q0:q0 + xq], xgt[:, q0:q0 + xq])
            h = hpool.tile([128, ib * cap], WDT)

            # ---- phase 1: hT blocks ----
            with tc.tile_pool(name="psA", bufs=2, space="PSUM") as psA:
                for b0, nb in groups1:
                    cw = db * 128
                    wt1 = w1pool.tile([128, 2 * cw], WDT, tag="w1")
                    nc.sync.dma_start(wt1[:, :nb * cw],
                                      w1d[:, b0 * cw:(b0 + nb) * cw])
                    wt3 = w3pool.tile([128, 2 * cw], WDT, tag="w3")
                    nc.sync.dma_start(wt3[:, :nb * cw],
                                      w3d[:, b0 * cw:(b0 + nb) * cw])
                    for s in range(nb):
                        b = b0 + s
                        ps1 = psA.tile([128, cap], F32, tag="ps1")
                        ps3 = psA.tile([128, cap], F32, tag="ps3")
                        for do in range(db):
                            lo = (s * db + do) * 128
                            nc.tensor.matmul(
                                ps1[:], wt1[:, lo:lo + 128],
                                xg[:, do * cap:(do + 1) * cap],
                                start=(do == 0), stop=(do == db - 1))
                        for do in range(db):
                            lo = (s * db + do) * 128
                            nc.tensor.matmul(
                                ps3[:], wt3[:, lo:lo + 128],
                                xg[:, do * cap:(do + 1) * cap],
                                start=(do == 0), stop=(do == db - 1))
                        sil = spool.tile([128, cap], F32, tag="sil")
                        nc.scalar.activation(
                            sil[:], ps1[:],
                            mybir.ActivationFunctionType.Silu)
                        nc.vector.tensor_mul(
                            h[:, b * cap:(b + 1) * cap], sil[:], ps3[:])

            # ---- phase 2: yT[d, c] = sum_b w2[b, d].T @ hT[b, c] ----
            # stationary = w2 128-column d-blocks (M=128 exact), moving = hT
            # (N=cap). Output tiles are yT blocks, accumulated over all i.
            # w2 groups: 8 i-blocks per DMA (1 MB) for full HBM efficiency
            gsz = 8
            groups2 = [(b0, min(gsz, ib - b0)) for b0 in range(0, ib, gsz)]
            with tc.tile_pool(name="psB", bufs=2, space="PSUM") as psB:
                for off, w in passes:
                    ndc = w // 128
                    base = ib * off
                    po = {}
                    for dc in range(ndc):
                        po[dc] = psB.tile([128, cap], F32, tag=f"yT{dc}",
                                          name=f"po{dc}")
                    for b0, nb in groups2:
                        wt2 = w2pool.tile([128, gsz * 512], WDT, tag="w2")
                        nc.gpsimd.dma_start(
                            wt2[:, :nb * w],
                            w2d[:, base + b0 * w:base + (b0 + nb) * w])
                        for s in range(nb):
                            b = b0 + s
                            for dc in range(ndc):
                                lo = s * w + dc * 128
                                nc.tensor.matmul(
                                    po[dc][:],
                                    wt2[:, lo:lo + 128],
                                    h[:, b * cap:(b + 1) * cap],
                                    start=(b == 0), stop=(b == ib - 1))
                    for dc in range(ndc):
                        ot = opool.tile([128, cap], F32, tag="ot")
                        nc.vector.tensor_copy(ot[:], po[dc][:])
                        nc.scalar.dma_start(
                            yt[off + dc * 128:off + dc * 128 + 128, :],
                            ot[:])

    nc.compile()
    _BUILD_CACHE[key] = nc
    return nc


def _pack_w13(wk, d=D, i_dim=I):
    """Host-side relayout of a [I, D] w1/w3 matrix into the pre-transposed
    phase-1 device layout: [di, b, do, i_in] flattened to [128, ib*db*128]."""
    db, ib = d // 128, i_dim // 128
    return np.ascontiguousarray(
        wk.reshape(ib, 128, db, 128).transpose(3, 0, 2, 1)
    ).reshape(128, ib * db * 128)


def _pack_w2(wk, passes, d=D, i_dim=I):
    """[I, D] -> [i_in, concat over passes of [b, w_ph]]: per-pass flat
    block-major layout so phase 2 can DMA any run of i-blocks as one big
    contiguous transfer."""
    ib = i_dim // 128
    cols = []
    w3d = wk.reshape(ib, 128, d)
    for off, w in passes:
        cols.append(np.ascontiguousarray(
            w3d[:, :, off:off + w].transpose(1, 0, 2)).reshape(128, ib * w))
    return np.ascontiguousarray(np.concatenate(cols, axis=1))


def _prepare(inputs):
    """Host routing + packing. Returns (nc, in_maps, scatter_info)."""
    x = np.asarray(inputs["x"])
    idx = np.asarray(inputs["expert_indices"])
    w1 = np.asarray(inputs["w1"])
    w2 = np.asarray(inputs["w2"])
    w3 = np.asarray(inputs["w3"])

    t, a = idx.shape
    d, i_dim = x.shape[1], w1.shape[1]
    db = d // 128
    passes = _phase2_passes(d)

    # ---- host routing (the "all-to-all"), deduped per (token, expert) ----
    flat = idx.reshape(-1).astype(np.int64)
    code = np.repeat(np.arange(t, dtype=np.int64), a) * E + flat
    ucode = np.unique(code)                      # unique (token, expert)
    ue, ut = ucode % E, ucode // E
    order = np.argsort(ue, kind="stable")        # group unique pairs by expert
    ucode_g = ucode[order]
    counts = np.bincount(ue, minlength=E)
    starts = np.concatenate([[0], np.cumsum(counts)])
    cap = max(128, int(-(-counts.max() // 4) * 4))
    assert cap <= 512, f"capacity {cap} > 512 unsupported"
    # lookup: (token*E + expert) -> row in the expert-grouped concat output
    lut = np.full(t * E, -1, np.int64)
    lut[ucode_g] = np.arange(len(ucode_g))

    nc = _build(cap, d, i_dim)

    x_bf = x.astype(BF16)
    in_maps = []
    for k in range(E):
        sel = (ucode_g[starts[k]:starts[k + 1]]) // E   # tokens for expert k
        xg = np.zeros((cap, d), BF16)
        xg[:len(sel)] = x_bf[sel]
        # [c, d] -> [di, do, c]
        xgt = np.ascontiguousarray(
            xg.T.reshape(db, 128, cap).transpose(1, 0, 2)
        ).reshape(128, db * cap)
        in_maps.append({
            "xgt": xgt,
            "w1d": _pack_w13(w1[k].astype(BF16), d, i_dim),
            "w3d": _pack_w13(w3[k].astype(BF16), d, i_dim),
            "w2d": _pack_w2(w2[k].astype(BF16), passes, d, i_dim),
        })

    return nc, in_maps, (t, a, d, code, lut, counts, starts)


def _scatter(results, scatter_info):
    t, a, d, code, lut, counts, starts = scatter_info
    rows = np.concatenate(
        [results[k]["yt"][:, :counts[k]].T for k in range(E)], axis=0)
    out_flat = rows[lut[code]]
    return np.ascontiguousarray(out_flat.reshape(t, a, d), np.float32)


def kernel(**inputs):
    from concourse.bass_utils import run_bass_kernel_spmd

    nc, in_maps, scatter_info = _prepare(inputs)
    res = run_bass_kernel_spmd(nc, in_maps, core_ids=list(range(N_CORES)))
    return _scatter(res.results, scatter_info)


# revision 12
# speedup vs baseline: 1.0126x; 1.0089x over previous
"""MoE ConditionalFeedForward kernel for 8 trn2 NeuronCores.

Strategy: paired expert parallelism with uniform weight streaming.

Routing: unique (token, expert) pairs are computed once (a token whose two
slots pick the same expert is deduped) and grouped by expert on the host.
Experts are paired large-count-with-small-count; the pair of cores (2g, 2g+1)
owns the expert pair's weights split in half along the intermediate dim I.
Both cores process ALL tokens of both experts against their I-half and emit
full-D f32 partial outputs; the host sums the two partials per expert. This
balances PE work across cores: effective capacity = (capA + capB) / 2 instead
of the max expert count.

Device program (segments s = A, B with capacities Ns; D=2048, IH = I/2):
  phase 1 (per i-block): hT[i,c] = silu(w1T.x) * (w3T.x)   (PE, d on partitions)
  phase 2 (per i-block): yT[d,c] += w2[i,d] * hT[i,c]      (PE, i on partitions)

The i-blocks are processed in chunks, with phase 2 of chunk c interleaved
after phase 1 of chunk c+1. This makes the three weight streams (w1, w3 on
the sync DMA queue; w2 on the gpsimd queue) flow at a near-constant combined
~300 GB/s for the whole kernel — there is no phase transition at which a
25 MB stream has to start cold, and no prefetch burst that starves the
just-in-time stream. Cross-chunk accumulation of yT happens in an SBUF f32
accumulator via DVE adds (PSUM can only hold 4 live output tiles next to
phase 1's 4). All weights/activations stream as bf16; PSUM accumulation, the
SBUF accumulator and the partial outputs are f32, so pairing adds no
quantization error.
"""

import numpy as np
import ml_dtypes

BF16 = ml_dtypes.bfloat16

T, A, E, D, I = 1024, 2, 8, 2048, 5632
N_CORES = 8
NG = E // 2            # core pairs / expert pairs
IH = I // 2            # i-rows per core
PAIRED = True

_BUILD_CACHE = {}


def _chunks(ib):
    """i-block chunks: a small first chunk so the phase-2 stream's first
    tranche has an early deadline the HBM can meet, 6-block chunks after
    (chunk size bounds live w2 SBUF tiles: 3 x 2-block units)."""
    sizes = [4]
    rem = ib - 4
    while rem > 0:
        g = min(6, rem)
        sizes.append(g)
        rem -= g
    out, b0 = [], 0
    for n in sizes:
        out.append((b0, n))
        b0 += n
    return out


def _ph1_groups(b0, nb, fast_start):
    """w1/w3 DMA groups (block runs) inside one chunk; the very first chunk
    uses 1-block groups up front so the first matmul's weights land fast."""
    sizes = [1, 1] if fast_start else []
    rem = nb - len(sizes)
    while rem > 0:
        g = min(2, rem)
        sizes.append(g)
        rem -= g
    out, b = [], b0
    for g in sizes:
        out.append((b, g))
        b += g
    return out


def _build(caps, ibs, d=D):
    """Build + compile the per-core program.

    caps/ibs: per-segment token capacity and i-block count. A segment is one
    expert's token set against this core's slice of that expert's weights.
    """
    key = (caps, ibs, d)
    if key in _BUILD_CACHE:
        return _BUILD_CACHE[key]

    import concourse.mybir as mybir
    import concourse.tile as tile
    from concourse import bacc

    dt = mybir.dt
    WDT = dt.bfloat16
    F32 = dt.float32

    db = d // 128          # d-chunks (contraction of phase 1)
    dcs = d // 128         # d-column blocks of the output
    cw = db * 128          # w1/w3 columns per i-block
    nseg = len(caps)
    cmx = max(caps)        # tagged tiles share one shape; slice per segment
    for cap in caps:
        assert cap % 4 == 0 and cap <= 512
    segoff = [0]
    for ib in ibs:
        segoff.append(segoff[-1] + ib)
    ib_tot = segoff[-1]

    nc = bacc.Bacc("TRN2", target_bir_lowering=False, debug=False,
                   num_devices=N_CORES)

    xs, ys = [], []
    for s, cap in enumerate(caps):
        xs.append(nc.dram_tensor(f"xgt{s}", [128, db * cap], WDT,
                                 kind="ExternalInput").ap())
        ys.append(nc.dram_tensor(f"yt{s}", [d, cap], F32,
                                 kind="ExternalOutput").ap())
    w1d = nc.dram_tensor("w1d", [128, ib_tot * cw], WDT,
                         kind="ExternalInput").ap()
    w3d = nc.dram_tensor("w3d", [128, ib_tot * cw], WDT,
                         kind="ExternalInput").ap()
    w2d = nc.dram_tensor("w2d", [128, ib_tot * d], WDT,
                         kind="ExternalInput").ap()

    with tile.TileContext(nc) as tc:
        with (
            tc.tile_pool(name="xpool", bufs=1) as xpool,
            tc.tile_pool(name="w1pool", bufs=6) as w1pool,
            tc.tile_pool(name="w3pool", bufs=6) as w3pool,
            tc.tile_pool(name="w2pool", bufs=4) as w2pool,
            tc.tile_pool(name="hpool", bufs=1) as hpool,
            tc.tile_pool(name="spool", bufs=2) as spool,
            tc.tile_pool(name="apool", bufs=1) as apool,
            tc.tile_pool(name="opool", bufs=4) as opool,
            tc.tile_pool(name="psA", bufs=2, space="PSUM") as psA,
            tc.tile_pool(name="psB", bufs=1, space="PSUM") as psB,
        ):
            xg, h = [], []
            for s, cap in enumerate(caps):
                xg.append(xpool.tile([128, db * cap], WDT, name=f"xg{s}"))
                h.append(hpool.tile([128, ibs[s] * cap], WDT, name=f"h{s}"))
                xq = max(1, db // 4) * cap
                for q0 in range(0, db * cap, xq):
                    nc.scalar.dma_start(xg[s][:, q0:q0 + xq],
                                        xs[s][:, q0:q0 + xq])

            acc = [None] * nseg

            def ph1(s, c):
                b0, nb = c
                cap = caps[s]
                for g0, gn in _ph1_groups(b0, nb, fast_start=(s, b0) == (0, 0)):
                    ga = segoff[s] * cw + g0 * cw
                    wt1 = w1pool.tile([128, 2 * cw], WDT, tag="w1")
                    nc.sync.dma_start(wt1[:, :gn * cw],
                                      w1d[:, ga:ga + gn * cw])
                    wt3 = w3pool.tile([128, 2 * cw], WDT, tag="w3")
                    nc.sync.dma_start(wt3[:, :gn * cw],
                                      w3d[:, ga:ga + gn * cw])
                    for si in range(gn):
                        b = g0 + si
                        ps1 = psA.tile([128, cmx], F32, tag="ps1")
                        ps3 = psA.tile([128, cmx], F32, tag="ps3")
                        for do in range(db):
                            lo = (si * db + do) * 128
                            nc.tensor.matmul(
                                ps1[:, :cap], wt1[:, lo:lo + 128],
                                xg[s][:, do * cap:(do + 1) * cap],
                                start=(do == 0), stop=(do == db - 1))
                        for do in range(db):
                            lo = (si * db + do) * 128
                            nc.tensor.matmul(
                                ps3[:, :cap], wt3[:, lo:lo + 128],
                                xg[s][:, do * cap:(do + 1) * cap],
                                start=(do == 0), stop=(do == db - 1))
                        sil = spool.tile([128, cmx], F32, tag="sil")
                        nc.scalar.activation(
                            sil[:, :cap], ps1[:, :cap],
                            mybir.ActivationFunctionType.Silu)
                        nc.vector.tensor_mul(
                            h[s][:, b * cap:(b + 1) * cap], sil[:, :cap],
                            ps3[:, :cap])

            def ph2(s, c):
                b0, nb = c
                cap = caps[s]
                first_chunk = b0 == 0
                last_chunk = b0 + nb == ibs[s]
                if first_chunk:
                    acc[s] = [apool.tile([128, cmx], F32, tag=f"acc{dc}",
                                         name=f"acc{s}_{dc}")
                              for dc in range(dcs)]
                # 2-block w2 units; each holds its blocks' full D columns
                units = []
                u0 = 0
                while u0 < nb:
                    un = min(2, nb - u0)
                    units.append((u0, un))
                    u0 += un
                wts = []
                for u0, un in units:
                    ga = (segoff[s] + b0 + u0) * d
                    wt2 = w2pool.tile([128, 2 * d], WDT, tag="w2")
                    nc.gpsimd.dma_start(wt2[:, :un * d],
                                        w2d[:, ga:ga + un * d])
                    wts.append(wt2)
                for dcg in range(0, dcs, 4):
                    po = {}
                    for dc in range(dcg, dcg + 4):
                        po[dc] = psB.tile([128, cmx], F32, tag=f"po{dc % 4}",
                                          name=f"po{s}_{dc}")
                    for ui, (u0, un) in enumerate(units):
                        for si in range(un):
                            b = b0 + u0 + si
                            for dc in range(dcg, dcg + 4):
                                nc.tensor.matmul(
                                    po[dc][:, :cap],
                                    wts[ui][:, si * d + dc * 128:
                                            si * d + dc * 128 + 128],
                                    h[s][:, b * cap:(b + 1) * cap],
                                    start=(b == b0), stop=(b == b0 + nb - 1))
                    for dc in range(dcg, dcg + 4):
                        if first_chunk and last_chunk:
                            ot = opool.tile([128, cmx], F32, tag="ot")
                            nc.vector.tensor_copy(ot[:, :cap], po[dc][:, :cap])
                            nc.scalar.dma_start(
                                ys[s][dc * 128:dc * 128 + 128, :],
                                ot[:, :cap])
                        elif first_chunk:
                            nc.vector.tensor_copy(acc[s][dc][:, :cap],
                                                  po[dc][:, :cap])
                        elif not last_chunk:
                            nc.vector.tensor_add(
                                acc[s][dc][:, :cap], acc[s][dc][:, :cap],
                                po[dc][:, :cap])
                        else:
                            ot = opool.tile([128, cmx], F32, tag="ot")
                            nc.vector.tensor_add(
                                ot[:, :cap], acc[s][dc][:, :cap],
                                po[dc][:, :cap])
                            nc.scalar.dma_start(
                                ys[s][dc * 128:dc * 128 + 128, :],
                                ot[:, :cap])

            # software-pipelined task order: ph2 of chunk k runs after ph1 of
            # chunk k+1, so phase 2 never waits on the silu/mul tail of its
            # own chunk, and w1/w3/w2 stream concurrently all kernel long.
            tasks = [(s, c) for s in range(nseg) for c in _chunks(ibs[s])]
            for k, t in enumerate(tasks):
                ph1(*t)
                if k >= 1:
                    ph2(*tasks[k - 1])
            ph2(*tasks[-1])

    nc.compile()
    _BUILD_CACHE[key] = nc
    return nc


def _pack_w13(wk, d=D):
    """[ih, d] -> [di, b, do, i_in] flattened to [128, ib*db*128]."""
    db, ib = d // 128, wk.shape[0] // 128
    return np.ascontiguousarray(
        wk.reshape(ib, 128, db, 128).transpose(3, 0, 2, 1)
    ).reshape(128, ib * db * 128)


def _pack_w2(wk, d=D):
    """[ih, d] -> [i_in, b, dcol] flattened to [128, ib*d] (block-major)."""
    ib = wk.shape[0] // 128
    return np.ascontiguousarray(
        wk.reshape(ib, 128, d).transpose(1, 0, 2)).reshape(128, ib * d)


def _prepare(inputs):
    x = np.asarray(inputs["x"])
    idx = np.asarray(inputs["expert_indices"])
    w1 = np.asarray(inputs["w1"])
    w2 = np.asarray(inputs["w2"])
    w3 = np.asarray(inputs["w3"])

    t, a = idx.shape
    d = x.shape[1]
    db = d // 128

    # ---- host routing, deduped per (token, expert) ----
    flat = idx.reshape(-1).astype(np.int64)
    code = np.repeat(np.arange(t, dtype=np.int64), a) * E + flat
    ucode = np.unique(code)
    ue = ucode % E
    order = np.argsort(ue, kind="stable")
    ucode_g = ucode[order]
    counts = np.bincount(ue, minlength=E)
    starts = np.concatenate([[0], np.cumsum(counts)])
    lut = np.full(t * E, -1, np.int64)
    lut[ucode_g] = np.arange(len(ucode_g))

    x_bf = x.astype(BF16)

    def pack_x(k, cap):
        sel = (ucode_g[starts[k]:starts[k + 1]]) // E
        xgp = np.zeros((cap, d), BF16)
        xgp[:len(sel)] = x_bf[sel]
        return np.ascontiguousarray(
            xgp.T.reshape(db, 128, cap).transpose(1, 0, 2)
        ).reshape(128, db * cap)

    def r4(n):
        return max(128, int(-(-n // 4) * 4))

    if PAIRED:
        by_count = np.argsort(-counts, kind="stable")
        pairs = [(int(by_count[g]), int(by_count[E - 1 - g]))
                 for g in range(NG)]
        caps = (r4(max(counts[p[0]] for p in pairs)),
                r4(max(counts[p[1]] for p in pairs)))
        ibs = (IH // 128, IH // 128)
        nc = _build(caps, ibs, d)
        in_maps = []
        for g in range(NG):
            ea, eb = pairs[g]
            xa, xb = pack_x(ea, caps[0]), pack_x(eb, caps[1])
            for hf in range(2):
                rows = slice(hf * IH, (hf + 1) * IH)
                in_maps.append({
                    "xgt0": xa,
                    "xgt1": xb,
                    "w1d": np.concatenate(
                        [_pack_w13(w1[ea][rows].astype(BF16), d),
                         _pack_w13(w1[eb][rows].astype(BF16), d)], axis=1),
                    "w3d": np.concatenate(
                        [_pack_w13(w3[ea][rows].astype(BF16), d),
                         _pack_w13(w3[eb][rows].astype(BF16), d)], axis=1),
                    "w2d": np.concatenate(
                        [_pack_w2(w2[ea][rows].astype(BF16), d),
                         _pack_w2(w2[eb][rows].astype(BF16), d)], axis=1),
                })
        return nc, in_maps, (t, a, d, code, lut, counts, starts, pairs)
    else:
        cap = r4(counts.max())
        nc = _build((cap,), (w1.shape[1] // 128,), d)
        in_maps = []
        for k in range(E):
            in_maps.append({
                "xgt0": pack_x(k, cap),
                "w1d": _pack_w13(w1[k].astype(BF16), d),
                "w3d": _pack_w13(w3[k].astype(BF16), d),
                "w2d": _pack_w2(w2[k].astype(BF16), d),
            })
        return nc, in_maps, (t, a, d, code, lut, counts, starts, None)


def _scatter(results, scatter_info):
    t, a, d, code, lut, counts, starts, pairs = scatter_info
    rows_by_expert = [None] * E
    if pairs is not None:
        for g in range(NG):
            ea, eb = pairs[g]
            ya = results[2 * g]["yt0"] + results[2 * g + 1]["yt0"]
            yb = results[2 * g]["yt1"] + results[2 * g + 1]["yt1"]
            rows_by_expert[ea] = ya[:, :counts[ea]].T
            rows_by_expert[eb] = yb[:, :counts[eb]].T
    else:
        for k in range(E):
            rows_by_expert[k] = results[k]["yt0"][:, :counts[k]].T
    rows = np.concatenate(rows_by_expert, axis=0)
    out_flat = rows[lut[code]]
    return np.ascontiguousarray(out_flat.reshape(t, a, d), np.float32)


def kernel(**inputs):
    from concourse.bass_utils import run_bass_kernel_spmd

    nc, in_maps, scatter_info = _prepare(inputs)
    res = run_bass_kernel_spmd(nc, in_maps, core_ids=list(range(N_CORES)))
    return _scatter(res.results, scatter_info)


# revision 15
# speedup vs baseline: 1.0829x; 1.0695x over previous
"""MoE ConditionalFeedForward kernel for 8 trn2 NeuronCores.

Strategy: paired expert parallelism with uniform weight streaming.

Routing: unique (token, expert) pairs are computed once (a token whose two
slots pick the same expert is deduped) and grouped by expert on the host.
Experts are paired large-count-with-small-count; the pair of cores (2g, 2g+1)
owns the expert pair's weights split in half along the intermediate dim I.
Both cores process ALL tokens of both experts against their I-half and emit
full-D f32 partial outputs; the host sums the two partials per expert. This
balances PE work across cores: effective capacity = (capA + capB) / 2 instead
of the max expert count.

Device program (segments s = A, B with capacities Ns; D=2048, IH = I/2):
  phase 1 (per i-block): hT[i,c] = silu(w1T.x) * (w3T.x)   (PE, d on partitions)
  phase 2 (per i-block): yT[d,c] += w2[i,d] * hT[i,c]      (PE, i on partitions)

The i-blocks are processed in chunks, with phase 2 of chunk c interleaved
after phase 1 of chunk c+1. This makes the three weight streams (w1, w3 on
the sync DMA queue; w2 on the gpsimd queue) flow at a near-constant combined
~300 GB/s for the whole kernel — there is no phase transition at which a
25 MB stream has to start cold, and no prefetch burst that starves the
just-in-time stream. Cross-chunk accumulation of yT happens in an SBUF f32
accumulator via DVE adds (PSUM can only hold 4 live output tiles next to
phase 1's 4). All weights/activations stream as bf16; PSUM accumulation, the
SBUF accumulator and the partial outputs are f32, so pairing adds no
quantization error.
"""

import numpy as np
import ml_dtypes

BF16 = ml_dtypes.bfloat16

T, A, E, D, I = 1024, 2, 8, 2048, 5632
N_CORES = 8
NG = E // 2            # core pairs / expert pairs
IH = I // 2            # i-rows per core
PAIRED = True

_BUILD_CACHE = {}


def _chunks(ib):
    """i-block chunks: a small first chunk so the phase-2 stream's first
    tranche has an early deadline the HBM can meet, 6-block chunks after
    (chunk size bounds live w2 SBUF tiles: 3 x 2-block units)."""
    sizes = [4]
    rem = ib - 4
    while rem > 0:
        g = min(6, rem)
        sizes.append(g)
        rem -= g
    out, b0 = [], 0
    for n in sizes:
        out.append((b0, n))
        b0 += n
    return out


def _ph1_groups(b0, nb, fast_start):
    """w1/w3 DMA groups (block runs) inside one chunk; the very first chunk
    uses 1-block groups up front so the first matmul's weights land fast."""
    sizes = [1, 1] if fast_start else []
    rem = nb - len(sizes)
    while rem > 0:
        g = min(2, rem)
        sizes.append(g)
        rem -= g
    out, b = [], b0
    for g in sizes:
        out.append((b, g))
        b += g
    return out


def _build(caps, ibs, d=D):
    """Build + compile the per-core program.

    caps/ibs: per-segment token capacity and i-block count. A segment is one
    expert's token set against this core's slice of that expert's weights.
    """
    key = (caps, ibs, d)
    if key in _BUILD_CACHE:
        return _BUILD_CACHE[key]

    import concourse.mybir as mybir
    import concourse.tile as tile
    from concourse import bacc

    dt = mybir.dt
    WDT = dt.bfloat16
    F32 = dt.float32

    db = d // 128          # d-chunks (contraction of phase 1)
    dcs = d // 128         # d-column blocks of the output
    cw = db * 128          # w1/w3 columns per i-block
    nseg = len(caps)
    cmx = max(caps)        # tagged tiles share one shape; slice per segment
    for cap in caps:
        assert cap % 4 == 0 and cap <= 512
    segoff = [0]
    for ib in ibs:
        segoff.append(segoff[-1] + ib)
    ib_tot = segoff[-1]

    nc = bacc.Bacc("TRN2", target_bir_lowering=False, debug=False,
                   num_devices=N_CORES)

    xs, ys = [], []
    for s, cap in enumerate(caps):
        xs.append(nc.dram_tensor(f"xgt{s}", [128, db * cap], WDT,
                                 kind="ExternalInput").ap())
        ys.append(nc.dram_tensor(f"yt{s}", [d, cap], F32,
                                 kind="ExternalOutput").ap())
    w1d = nc.dram_tensor("w1d", [128, ib_tot * cw], WDT,
                         kind="ExternalInput").ap()
    w3d = nc.dram_tensor("w3d", [128, ib_tot * cw], WDT,
                         kind="ExternalInput").ap()
    w2d = nc.dram_tensor("w2d", [128, ib_tot * d], WDT,
                         kind="ExternalInput").ap()

    with tile.TileContext(nc) as tc:
        with (
            tc.tile_pool(name="xpool", bufs=1) as xpool,
            tc.tile_pool(name="w1pool", bufs=5) as w1pool,
            tc.tile_pool(name="w3pool", bufs=5) as w3pool,
            tc.tile_pool(name="w2pool", bufs=6) as w2pool,
            tc.tile_pool(name="hpool", bufs=1) as hpool,
            tc.tile_pool(name="spool", bufs=2) as spool,
            tc.tile_pool(name="apool", bufs=1) as apool,
            tc.tile_pool(name="opool", bufs=4) as opool,
            tc.tile_pool(name="psA", bufs=2, space="PSUM") as psA,
            tc.tile_pool(name="psB", bufs=1, space="PSUM") as psB,
        ):
            xg, h = [], []
            for s, cap in enumerate(caps):
                xg.append(xpool.tile([128, db * cap], WDT, name=f"xg{s}"))
                h.append(hpool.tile([128, ibs[s] * cap], WDT, name=f"h{s}"))
                xq = max(1, db // 4) * cap
                for q0 in range(0, db * cap, xq):
                    nc.scalar.dma_start(xg[s][:, q0:q0 + xq],
                                        xs[s][:, q0:q0 + xq])

            acc = [None] * nseg

            def ph1(s, c):
                b0, nb = c
                cap = caps[s]
                for g0, gn in _ph1_groups(b0, nb, fast_start=(s, b0) == (0, 0)):
                    ga = segoff[s] * cw + g0 * cw
                    wt1 = w1pool.tile([128, 2 * cw], WDT, tag="w1")
                    nc.sync.dma_start(wt1[:, :gn * cw],
                                      w1d[:, ga:ga + gn * cw])
                    wt3 = w3pool.tile([128, 2 * cw], WDT, tag="w3")
                    nc.sync.dma_start(wt3[:, :gn * cw],
                                      w3d[:, ga:ga + gn * cw])
                    for si in range(gn):
                        b = g0 + si
                        ps1 = psA.tile([128, cmx], F32, tag="ps1")
                        ps3 = psA.tile([128, cmx], F32, tag="ps3")
                        for do in range(db):
                            lo = (si * db + do) * 128
                            nc.tensor.matmul(
                                ps1[:, :cap], wt1[:, lo:lo + 128],
                                xg[s][:, do * cap:(do + 1) * cap],
                                start=(do == 0), stop=(do == db - 1))
                        for do in range(db):
                            lo = (si * db + do) * 128
                            nc.tensor.matmul(
                                ps3[:, :cap], wt3[:, lo:lo + 128],
                                xg[s][:, do * cap:(do + 1) * cap],
                                start=(do == 0), stop=(do == db - 1))
                        sil = spool.tile([128, cmx], F32, tag="sil")
                        nc.scalar.activation(
                            sil[:, :cap], ps1[:, :cap],
                            mybir.ActivationFunctionType.Silu)
                        nc.vector.tensor_mul(
                            h[s][:, b * cap:(b + 1) * cap], sil[:, :cap],
                            ps3[:, :cap])

            def ph2(s, c):
                b0, nb = c
                cap = caps[s]
                first_chunk = b0 == 0
                last_chunk = b0 + nb == ibs[s]
                if first_chunk:
                    acc[s] = [apool.tile([128, cmx], F32, tag=f"acc{dc}",
                                         name=f"acc{s}_{dc}")
                              for dc in range(dcs)]
                # 2-block w2 units; each holds its blocks' full D columns.
                # The kernel's first two chunks use 1-block units so the w2
                # stream's initial fill ramps up instead of bursting while
                # the w1/w3 stream is still at zero lead.
                usz = 1 if (s == 0 and b0 <= 4) else 2
                units = []
                u0 = 0
                while u0 < nb:
                    un = min(usz, nb - u0)
                    units.append((u0, un))
                    u0 += un
                wts = []
                for u0, un in units:
                    ga = (segoff[s] + b0 + u0) * d
                    wt2 = w2pool.tile([128, 2 * d], WDT, tag="w2")
                    nc.gpsimd.dma_start(wt2[:, :un * d],
                                        w2d[:, ga:ga + un * d])
                    wts.append(wt2)
                for dcg in range(0, dcs, 4):
                    po = {}
                    for dc in range(dcg, dcg + 4):
                        po[dc] = psB.tile([128, cmx], F32, tag=f"po{dc % 4}",
                                          name=f"po{s}_{dc}")
                    for ui, (u0, un) in enumerate(units):
                        for si in range(un):
                            b = b0 + u0 + si
                            for dc in range(dcg, dcg + 4):
                                nc.tensor.matmul(
                                    po[dc][:, :cap],
                                    wts[ui][:, si * d + dc * 128:
                                            si * d + dc * 128 + 128],
                                    h[s][:, b * cap:(b + 1) * cap],
                                    start=(b == b0), stop=(b == b0 + nb - 1))
                    for dc in range(dcg, dcg + 4):
                        if first_chunk and last_chunk:
                            ot = opool.tile([128, cmx], F32, tag="ot")
                            nc.vector.tensor_copy(ot[:, :cap], po[dc][:, :cap])
                            nc.sync.dma_start(
                                ys[s][dc * 128:dc * 128 + 128, :],
                                ot[:, :cap])
                        elif first_chunk:
                            nc.vector.tensor_copy(acc[s][dc][:, :cap],
                                                  po[dc][:, :cap])
                        elif not last_chunk:
                            nc.vector.tensor_add(
                                acc[s][dc][:, :cap], acc[s][dc][:, :cap],
                                po[dc][:, :cap])
                        else:
                            ot = opool.tile([128, cmx], F32, tag="ot")
                            nc.vector.tensor_add(
                                ot[:, :cap], acc[s][dc][:, :cap],
                                po[dc][:, :cap])
                            nc.sync.dma_start(
                                ys[s][dc * 128:dc * 128 + 128, :],
                                ot[:, :cap])

            # software-pipelined task order: ph2 of chunk k runs after ph1 of
            # chunk k+1, so phase 2 never waits on the silu/mul tail of its
            # own chunk, and w1/w3/w2 stream concurrently all kernel long.
            tasks = [(s, c) for s in range(nseg) for c in _chunks(ibs[s])]
            for k, t in enumerate(tasks):
                ph1(*t)
                if k >= 1:
                    ph2(*tasks[k - 1])
            ph2(*tasks[-1])

    nc.compile()
    _BUILD_CACHE[key] = nc
    return nc


def _pack_w13(wk, d=D):
    """[ih, d] -> [di, b, do, i_in] flattened to [128, ib*db*128]."""
    db, ib = d // 128, wk.shape[0] // 128
    return np.ascontiguousarray(
        wk.reshape(ib, 128, db, 128).transpose(3, 0, 2, 1)
    ).reshape(128, ib * db * 128)


def _pack_w2(wk, d=D):
    """[ih, d] -> [i_in, b, dcol] flattened to [128, ib*d] (block-major)."""
    ib = wk.shape[0] // 128
    return np.ascontiguousarray(
        wk.reshape(ib, 128, d).transpose(1, 0, 2)).reshape(128, ib * d)


def _prepare(inputs):
    x = np.asarray(inputs["x"])
    idx = np.asarray(inputs["expert_indices"])
    w1 = np.asarray(inputs["w1"])
    w2 = np.asarray(inputs["w2"])
    w3 = np.asarray(inputs["w3"])

    t, a = idx.shape
    d = x.shape[1]
    db = d // 128

    # ---- host routing, deduped per (token, expert) ----
    flat = idx.reshape(-1).astype(np.int64)
    code = np.repeat(np.arange(t, dtype=np.int64), a) * E + flat
    ucode = np.unique(code)
    ue = ucode % E
    order = np.argsort(ue, kind="stable")
    ucode_g = ucode[order]
    counts = np.bincount(ue, minlength=E)
    starts = np.concatenate([[0], np.cumsum(counts)])
    lut = np.full(t * E, -1, np.int64)
    lut[ucode_g] = np.arange(len(ucode_g))

    x_bf = x.astype(BF16)

    def pack_x(k, cap):
        sel = (ucode_g[starts[k]:starts[k + 1]]) // E
        xgp = np.zeros((cap, d), BF16)
        xgp[:len(sel)] = x_bf[sel]
        return np.ascontiguousarray(
            xgp.T.reshape(db, 128, cap).transpose(1, 0, 2)
        ).reshape(128, db * cap)

    def r4(n):
        return max(128, int(-(-n // 4) * 4))

    if PAIRED:
        by_count = np.argsort(-counts, kind="stable")
        pairs = [(int(by_count[g]), int(by_count[E - 1 - g]))
                 for g in range(NG)]
        caps = (r4(max(counts[p[0]] for p in pairs)),
                r4(max(counts[p[1]] for p in pairs)))
        ibs = (IH // 128, IH // 128)
        nc = _build(caps, ibs, d)
        in_maps = []
        for g in range(NG):
            ea, eb = pairs[g]
            xa, xb = pack_x(ea, caps[0]), pack_x(eb, caps[1])
            for hf in range(2):
                rows = slice(hf * IH, (hf + 1) * IH)
                in_maps.append({
                    "xgt0": xa,
                    "xgt1": xb,
                    "w1d": np.concatenate(
                        [_pack_w13(w1[ea][rows].astype(BF16), d),
                         _pack_w13(w1[eb][rows].astype(BF16), d)], axis=1),
                    "w3d": np.concatenate(
                        [_pack_w13(w3[ea][rows].astype(BF16), d),
                         _pack_w13(w3[eb][rows].astype(BF16), d)], axis=1),
                    "w2d": np.concatenate(
                        [_pack_w2(w2[ea][rows].astype(BF16), d),
                         _pack_w2(w2[eb][rows].astype(BF16), d)], axis=1),
                })
        return nc, in_maps, (t, a, d, code, lut, counts, starts, pairs)
    else:
        cap = r4(counts.max())
        nc = _build((cap,), (w1.shape[1] // 128,), d)
        in_maps = []
        for k in range(E):
            in_maps.append({
                "xgt0": pack_x(k, cap),
                "w1d": _pack_w13(w1[k].astype(BF16), d),
                "w3d": _pack_w13(w3[k].astype(BF16), d),
                "w2d": _pack_w2(w2[k].astype(BF16), d),
            })
        return nc, in_maps, (t, a, d, code, lut, counts, starts, None)


def _scatter(results, scatter_info):
    t, a, d, code, lut, counts, starts, pairs = scatter_info
    rows_by_expert = [None] * E
    if pairs is not None:
        for g in range(NG):
            ea, eb = pairs[g]
            ya = results[2 * g]["yt0"] + results[2 * g + 1]["yt0"]
            yb = results[2 * g]["yt1"] + results[2 * g + 1]["yt1"]
            rows_by_expert[ea] = ya[:, :counts[ea]].T
            rows_by_expert[eb] = yb[:, :counts[eb]].T
    else:
        for k in range(E):
            rows_by_expert[k] = results[k]["yt0"][:, :counts[k]].T
    rows = np.concatenate(rows_by_expert, axis=0)
    out_flat = rows[lut[code]]
    return np.ascontiguousarray(out_flat.reshape(t, a, d), np.float32)


def kernel(**inputs):
    from concourse.bass_utils import run_bass_kernel_spmd

    nc, in_maps, scatter_info = _prepare(inputs)
    res = run_bass_kernel_spmd(nc, in_maps, core_ids=list(range(N_CORES)))
    return _scatter(res.results, scatter_info)
